# revision 1
# baseline (speedup 1.0000x reference)
"""GPT (4-layer, C=512, H=8, T=1024, B=2, V=50257, LoRA r=8) on 8 trn2 cores.

Sharding: 2 groups of 4 cores (one per batch element); sequence-parallel
within a group (each core owns 256 tokens) with a per-layer KV AllGather;
vocab-sharded head matmul after a final all-core AllGather of x.
SPMD-uniform program: rank differences live in host-side data (causal masks,
padded head shards).
"""
import math
import numpy as np
import ml_dtypes

import concourse.bass as bass
import concourse.bacc as bacc
import concourse.tile as tile
import concourse.mybir as mybir
from concourse import bass_utils

BF16 = mybir.dt.bfloat16
F32 = mybir.dt.float32
AF = mybir.ActivationFunctionType

L, H, C, V, B, T = 4, 8, 512, 50257, 2, 1024
R = 8
NCORES = 8
TO = 256            # tokens owned per core
NTT = TO // 128     # 2 token tiles per core
NF = C // 128       # 4 feature tiles
HD = C // H         # 64 head dim
VC = 6283           # padded vocab shard (8*6283 = 50264 >= 50257)
NVCH = 13           # vocab chunks of 512 (last = 139)
NEG = -1.0e9

_CACHE = {}


def build_nc(debug=False, do_layers=True, do_head=True, reps=1,
             no_coll=False, psum_dma=False):
    nc = bacc.Bacc("TRN2", target_bir_lowering=False, debug=False,
                   num_devices=NCORES)
    d = {}
    def inp(name, shape, dt):
        d[name] = nc.dram_tensor(name, shape, dt, kind="ExternalInput").ap()
    inp("x0", [NTT, 128, C], F32)
    inp("masks", [8, 128, TO], F32)
    inp("ident", [128, 128], BF16)
    inp("aw", [L, C, 3 * C], BF16)     # attn_w.T, q-cols pre-scaled
    inp("ala", [L, C, R], BF16)
    inp("alb", [L, R, 3 * C], BF16)    # *4.0, q-cols pre-scaled
    inp("pw", [L, C, C], BF16)
    inp("pla", [L, C, R], BF16)
    inp("plb", [L, R, C], BF16)        # *4.0
    inp("fw", [L, C, 4 * C], BF16)
    inp("mw", [L, 4 * C, C], BF16)
    inp("hw", [C, VC], BF16)           # head shard (rank-dep, zero-padded)
    y_d = nc.dram_tensor("y", [8 * TO, VC], F32, kind="ExternalOutput").ap()
    if debug:
        xdbg = nc.dram_tensor("xdbg", [L, NTT, 128, C], F32,
                              kind="ExternalOutput").ap()

    with tile.TileContext(nc) as tc:
        with (
            tc.tile_pool(name="persist", bufs=1) as pp,
            tc.tile_pool(name="wts", bufs=1) as wp,
            tc.tile_pool(name="acts", bufs=1) as ap_,
            tc.tile_pool(name="acts3", bufs=3) as ap3,
            tc.tile_pool(name="stats", bufs=3) as sp,
            tc.tile_pool(name="dram", bufs=2, space="DRAM") as dp,
            tc.tile_pool(name="psu", bufs=8, space="PSUM") as psu,
        ):
            ident = pp.tile([128, 128], BF16)
            nc.sync.dma_start(ident[:], d["ident"][:])
            zt = pp.tile([128, 1], F32)
            nc.vector.memset(zt[:], 0.0)
            eps = pp.tile([128, 1], F32)
            nc.vector.memset(eps[:], 1e-5)
            maskT = pp.tile([128, 8, TO], F32)
            nc.sync.dma_start(maskT[:], d["masks"].rearrange("k p q -> p k q"))

            x = [pp.tile([128, C], F32, name=f"x{tt}", tag=f"x{tt}") for tt in range(NTT)]

            kt_all = [pp.tile([128, T], BF16, name=f"kt{f}", tag=f"kt{f}") for f in range(NF)]
            v_aug = [pp.tile([128, H, HD + 1], BF16, name=f"va{kb}", tag=f"va{kb}")
                     for kb in range(T // 128)]
            for kb in range(T // 128):
                nc.vector.memset(v_aug[kb][:, :, HD:HD + 1], 1.0)

            def layernorm(src_tiles, eng_alt):
                """Return bf16 normalized tiles (gamma folded on host, beta==0)."""
                out = []
                for tt in range(NTT):
                    nm = sp.tile([128, 1], F32, name="nm", tag="nm")
                    nc.vector.reduce_sum(nm[:], src_tiles[tt][:],
                                         axis=mybir.AxisListType.X, negate=True)
                    nms = sp.tile([128, 1], F32, name="nms", tag="nms")
                    nc.vector.tensor_scalar_mul(nms[:], nm[:], 1.0 / C)
                    xc = ap_.tile([128, C], F32, name="xc", tag="xc")
                    nc.vector.tensor_scalar_add(xc[:], src_tiles[tt][:], nms[:])
                    sq = ap_.tile([128, C], BF16, name="sq", tag="sq")
                    ssq = sp.tile([128, 1], F32, name="ssq", tag="ssq")
                    nc.scalar.activation(sq[:], xc[:], AF.Square,
                                         bias=zt[:], accum_out=ssq[:])
                    std = sp.tile([128, 1], F32, name="std", tag="std")
                    nc.scalar.activation(std[:], ssq[:], AF.Sqrt,
                                         bias=eps[:], scale=1.0 / C)
                    rstd = sp.tile([128, 1], F32, name="rstd", tag="rstd")
                    nc.vector.reciprocal(rstd[:], std[:])
                    hb = ap_.tile([128, C], BF16, name=f"h{tt}", tag=f"h{tt}")
                    nc.vector.tensor_scalar_mul(hb[:], xc[:], rstd[:])
                    out.append(hb)
                return out

            def transpose_128(src_ap, dst_ap, eng):
                ptr = psu.tile([128, 128], BF16, name="tr", tag="u")
                nc.tensor.transpose(ptr[:], src_ap, ident[:])
                if eng == 0:
                    nc.scalar.copy(dst_ap, ptr[:])
                else:
                    nc.vector.tensor_copy(dst_ap, ptr[:])

            def transpose_tiles(tiles, nfree, tag):
                """tiles: list of [128, nfree*128] (token-major) ->
                list of nfree tiles [128, len(tiles)*128] (feature-major)."""
                outs = [ap_.tile([128, len(tiles) * 128], BF16, name=f"{tag}{f}", tag=f"{tag}{f}")
                        for f in range(nfree)]
                e = 0
                for i, t in enumerate(tiles):
                    for f in range(nfree):
                        transpose_128(t[:, f * 128:(f + 1) * 128],
                                      outs[f][:, i * 128:(i + 1) * 128], e % 2)
                        e += 1
                return outs

            for _rep in range(reps):
                for tt in range(NTT):
                    nc.sync.dma_start(x[tt][:], d["x0"][tt])
                for li in range(L if do_layers else 0):
                    aw = wp.tile([128, NF, 3 * C], BF16, name="aw", tag="aw", bufs=2)
                    nc.sync.dma_start(aw[:], d["aw"][li].rearrange(
                        "(f p) n -> p f n", p=128))
                    ala = wp.tile([128, NF, R], BF16, name="ala", tag="ala")
                    nc.sync.dma_start(ala[:], d["ala"][li].rearrange(
                        "(f p) n -> p f n", p=128))
                    alb = wp.tile([R, 3 * C], BF16, name="alb", tag="alb")
                    nc.sync.dma_start(alb[:], d["alb"][li])
                    pw = wp.tile([128, NF, C], BF16, name="pw", tag="pw")
                    nc.sync.dma_start(pw[:], d["pw"][li].rearrange(
                        "(f p) n -> p f n", p=128))
                    pla = wp.tile([128, NF, R], BF16, name="pla", tag="pla")
                    nc.sync.dma_start(pla[:], d["pla"][li].rearrange(
                        "(f p) n -> p f n", p=128))
                    plb = wp.tile([R, C], BF16, name="plb", tag="plb")
                    nc.sync.dma_start(plb[:], d["plb"][li])
                    fw = wp.tile([128, NF, 4 * C], BF16, name="fw", tag="fw", bufs=2)
                    nc.sync.dma_start(fw[:], d["fw"][li].rearrange(
                        "(f p) n -> p f n", p=128))
                    mw = wp.tile([128, 16, C], BF16, name="mw", tag="mw", bufs=2)
                    nc.sync.dma_start(mw[:], d["mw"][li].rearrange(
                        "(f p) n -> p f n", p=128))

                    # ---- attention ----
                    h = layernorm(x, 0)
                    hT = transpose_tiles(h, NF, "hT")

                    # LoRA down-proj: z [R, TO]
                    pz = psu.tile([R, TO], F32, name="z", tag="u")
                    for f in range(NF):
                        nc.tensor.matmul(pz[:], ala[:, f, :], hT[f][:],
                                         start=(f == 0), stop=(f == NF - 1))
                    z = sp.tile([R, TO], BF16, name="z", tag="z")
                    nc.scalar.copy(z[:], pz[:])

                    # qkvT [3C, TO] feature-major, 12 chunks of 128
                    qT = [ap_.tile([128, TO], BF16, name=f"qT{f}", tag=f"qT{f}") for f in range(NF)]
                    vT = [ap_.tile([128, TO], BF16, name=f"vT{f}", tag=f"vT{f}") for f in range(NF)]
                    kt_own = ap_.tile([128, NF, TO], BF16, name="kt_own", tag="kt_own")
                    for ch in range(3 * NF):
                        pq = psu.tile([128, TO], F32, name="mm256", tag="u")
                        for f in range(NF):
                            nc.tensor.matmul(pq[:], aw[:, f, ch * 128:(ch + 1) * 128],
                                             hT[f][:], start=(f == 0), stop=False)
                        nc.tensor.matmul(pq[:], alb[:, ch * 128:(ch + 1) * 128],
                                         z[:], start=False, stop=True)
                        if ch < NF:
                            nc.scalar.copy(qT[ch][:], pq[:])
                        elif ch < 2 * NF:
                            nc.scalar.copy(kt_own[:, ch - NF, :], pq[:])
                        else:
                            nc.vector.tensor_copy(vT[ch - 2 * NF][:], pq[:])

                    # V own -> token-major [TO, C] stored flat [128, (t c)]
                    v_own = ap_.tile([128, NTT * C], BF16, name="v_own",
                                     tag="v_own")
                    e = 0
                    for f in range(NF):
                        for t in range(NTT):
                            transpose_128(vT[f][:, t * 128:(t + 1) * 128],
                                          v_own[:, t * C + f * 128:
                                                t * C + (f + 1) * 128], e % 2)
                            e += 1

                    # KV AllGather within group of 4
                    cin = dp.tile([2, 128, NF, TO], BF16, name="cin", tag="cin")
                    cout = dp.tile([4, 2, 128, NF, TO], BF16, name="cout",
                                   tag="cout")
                    nc.sync.dma_start(cin[0], kt_own[:])
                    nc.sync.dma_start(
                        cin[1], v_own[:].rearrange("p (f q) -> p f q", f=NF))
                    nc.gpsimd.collective_compute(
                        "AllGather", mybir.AluOpType.bypass,
                        ins=[cin.opt()], outs=[cout.opt()],
                        replica_groups=[[0, 1, 2, 3], [4, 5, 6, 7]],
                    )
                    for r in range(4):
                        for f in range(NF):
                            nc.sync.dma_start(kt_all[f][:, r * TO:(r + 1) * TO],
                                              cout[r, 0, :, f, :])
                        for t in range(NTT):
                            kb = 2 * r + t
                            nc.sync.dma_start(
                                v_aug[kb][:, :, 0:HD],
                                cout[r, 1, :, 2 * t:2 * t + 2, :].rearrange(
                                    "p a (x e) -> p (a x) e", x=NF))

                    # attention: ST [k, q] per head per k-chunk; P=exp; AV
                    y_sb = [ap_.tile([128, C], BF16, name=f"y{tt}", tag=f"y{tt}")
                            for tt in range(NTT)]
                    pyav = [[None] * 2 for _ in range(NTT)]
                    for hh in range(H):
                        f, po = hh // 2, (hh % 2) * HD
                        pts = []
                        for kp in range(T // 256):
                            pst = psu.tile([128, 2, TO], F32, name="mm256",
                                           tag="u")
                            for j in range(2):
                                kc = 2 * kp + j
                                nc.tensor.matmul(
                                    pst[:, j, :],
                                    kt_all[f][po:po + HD,
                                              kc * 128:(kc + 1) * 128],
                                    qT[f][po:po + HD, :],
                                    start=True, stop=True)
                            nc.vector.tensor_add(
                                pst[:], pst[:],
                                maskT[:, 2 * kp:2 * kp + 2, :])
                            pt = ap3.tile([128, 2, TO], BF16, name="pt",
                                          tag="pt", bufs=4)
                            nc.scalar.activation(pt[:], pst[:], AF.Exp,
                                                 bias=zt[:])
                            pts.append(pt)
                        for tt in range(NTT):
                            hb, hi = hh // 4, hh % 4
                            if hi == 0:
                                pyav[tt][hb] = psu.tile([128, 4, HD + 1], F32,
                                                          name="yav", tag="u")
                            for kc in range(T // 128):
                                nc.tensor.matmul(
                                    pyav[tt][hb][:, hi, :],
                                    pts[kc // 2][:, kc % 2,
                                                 tt * 128:(tt + 1) * 128],
                                    v_aug[kc][:, hh, :],
                                    start=(kc == 0), stop=(kc == T // 128 - 1))
                            if hi == 3:
                                sums = sp.tile([128, 4], F32, name="sums", tag="sums")
                                for j in range(4):
                                    nc.scalar.copy(sums[:, j:j + 1],
                                                   pyav[tt][hb][:, j, HD:HD + 1])
                                rec = sp.tile([128, 4], F32, name="rec", tag="rec")
                                nc.vector.reciprocal(rec[:], sums[:])
                                for j in range(4):
                                    hj = hb * 4 + j
                                    nc.vector.tensor_scalar_mul(
                                        y_sb[tt][:, hj * HD:(hj + 1) * HD],
                                        pyav[tt][hb][:, j, 0:HD],
                                        rec[:, j:j + 1])

                    # proj + LoRA + residual
                    yT = transpose_tiles(y_sb, NF, "yT")
                    pz2 = psu.tile([R, TO], F32, name="z", tag="u")
                    for f in range(NF):
                        nc.tensor.matmul(pz2[:], pla[:, f, :], yT[f][:],
                                         start=(f == 0), stop=(f == NF - 1))
                    z2 = sp.tile([R, TO], BF16, name="z", tag="z")
                    nc.scalar.copy(z2[:], pz2[:])
                    for tt in range(NTT):
                        pp_ = psu.tile([128, C], F32, name="mm512", tag="u")
                        for f in range(NF):
                            nc.tensor.matmul(pp_[:], yT[f][:, tt * 128:(tt + 1) * 128],
                                             pw[:, f, :], start=(f == 0), stop=False)
                        nc.tensor.matmul(pp_[:], z2[:, tt * 128:(tt + 1) * 128],
                                         plb[:], start=False, stop=True)
                        nc.vector.tensor_add(x[tt][:], x[tt][:], pp_[:])

                    # ---- MLP ----
                    h2 = layernorm(x, 1)
                    h2T = transpose_tiles(h2, NF, "h2T")
                    # fc output computed feature-major: mF[g] = [128, TO],
                    # g-th 128-slice of the 2048 hidden dim; no transposes.
                    mF = [ap_.tile([128, TO], BF16, name=f"mF{g}", tag=f"mF{g}")
                          for g in range(16)]
                    for gp in range(8):
                        pf = psu.tile([128, 2, TO], F32, name="mm512", tag="u")
                        for j in range(2):
                            g = 2 * gp + j
                            for f in range(NF):
                                nc.tensor.matmul(
                                    pf[:, j, :],
                                    fw[:, f, g * 128:(g + 1) * 128],
                                    h2T[f][:],
                                    start=(f == 0), stop=(f == NF - 1))
                        nc.scalar.activation(mF[2 * gp][:], pf[:, 0, :],
                                             AF.Gelu_apprx_tanh, bias=zt[:])
                        nc.scalar.activation(mF[2 * gp + 1][:], pf[:, 1, :],
                                             AF.Gelu_apprx_tanh, bias=zt[:])
                    for tt in range(NTT):
                        pm = psu.tile([128, C], F32, name="mm512", tag="u")
                        for f in range(16):
                            nc.tensor.matmul(pm[:], mF[f][:, tt * 128:(tt + 1) * 128],
                                             mw[:, f, :],
                                             start=(f == 0), stop=(f == 15))
                        nc.vector.tensor_add(x[tt][:], x[tt][:], pm[:])
                        if debug:
                            nc.sync.dma_start(xdbg[li, tt], x[tt][:])

                if do_head:
                    # ---- final LN + all-core AllGather of xT + head ----
                    xf = layernorm(x, 0)
                    xfT = transpose_tiles(xf, NF, "xfT")
                    cinF = dp.tile([NF, 128, TO], BF16, name="cinF", tag="cinF")
                    coutF = dp.tile([8, NF, 128, TO], BF16, name="coutF", tag="coutF",
                                     addr_space="Shared")
                    for f in range(NF):
                        nc.sync.dma_start(cinF[f], xfT[f][:])
                    nc.gpsimd.collective_compute(
                        "AllGather", mybir.AluOpType.bypass,
                        ins=[cinF.opt()], outs=[coutF.opt()],
                        replica_groups=[[0, 1, 2, 3, 4, 5, 6, 7]],
                    )
                    xT_all = [pp.tile([128, 8 * TO], BF16, name=f"xta{f}", tag=f"xta{f}")
                              for f in range(NF)]
                    for r in range(8):
                        for f in range(NF):
                            nc.sync.dma_start(xT_all[f][:, r * TO:(r + 1) * TO],
                                              coutF[r, f])
                    for ch in range(NVCH):
                        nch = min(512, VC - ch * 512)
                        hwt = wp.tile([128, NF, 512], BF16, name="hw", tag="hw", bufs=2)
                        nc.sync.dma_start(
                            hwt[:, :, 0:nch],
                            d["hw"][:, ch * 512:ch * 512 + nch].rearrange(
                                "(f p) n -> p f n", p=128))
                        for tt in range(16):
                            pl = psu.tile([128, 512], F32, name="mm512", tag="u")
                            for f in range(NF):
                                nc.tensor.matmul(
                                    pl[:, 0:nch],
                                    xT_all[f][:, tt * 128:(tt + 1) * 128],
                                    hwt[:, f, 0:nch],
                                    start=(f == 0), stop=(f == NF - 1))
                            lo = ap3.tile([128, 512], F32, name="lo", tag="lo")
                            if tt % 2 == 0:
                                nc.scalar.copy(lo[:, 0:nch], pl[:, 0:nch])
                            else:
                                nc.vector.tensor_copy(lo[:, 0:nch], pl[:, 0:nch])
                            nc.sync.dma_start(
                                y_d[tt * 128:(tt + 1) * 128,
                                    ch * 512:ch * 512 + nch],
                                lo[:, 0:nch])

    nc.compile()
    return nc


def _bf(a):
    return np.ascontiguousarray(a.astype(ml_dtypes.bfloat16))


def host_shards(inputs, debug=False):
    idx = np.asarray(inputs["idx"])
    wte = np.asarray(inputs["wte"], np.float32)
    wpe = np.asarray(inputs["wpe"], np.float32)
    ln1_g = np.asarray(inputs["ln1_g"], np.float32)
    ln2_g = np.asarray(inputs["ln2_g"], np.float32)
    lnf_g = np.asarray(inputs["lnf_g"], np.float32)
    for nm in ("ln1_b", "ln2_b", "fc_b", "mproj_b", "lnf_b"):
        assert np.abs(np.asarray(inputs[nm])).max() == 0.0, f"{nm} nonzero"
    LS = 32.0 / 8.0
    qs = 1.0 / math.sqrt(HD)

    aw = np.empty((L, C, 3 * C), np.float32)
    ala = np.empty((L, C, R), np.float32)
    alb = np.empty((L, R, 3 * C), np.float32)
    pw = np.empty((L, C, C), np.float32)
    pla = np.empty((L, C, R), np.float32)
    plb = np.empty((L, R, C), np.float32)
    fw = np.empty((L, C, 4 * C), np.float32)
    mw = np.empty((L, 4 * C, C), np.float32)
    for i in range(L):
        a = (np.asarray(inputs["attn_w"][i], np.float32) * ln1_g[i][None, :]).T
        a = a.copy()
        a[:, :C] *= qs
        aw[i] = a
        ala[i] = (np.asarray(inputs["attn_lA"][i], np.float32)
                  * ln1_g[i][None, :]).T
        b = np.asarray(inputs["attn_lB"][i], np.float32).T * LS
        b = b.copy()
        b[:, :C] *= qs
        alb[i] = b
        pw[i] = np.asarray(inputs["proj_w"][i], np.float32).T
        pla[i] = np.asarray(inputs["proj_lA"][i], np.float32).T
        plb[i] = np.asarray(inputs["proj_lB"][i], np.float32).T * LS
        fw[i] = (np.asarray(inputs["fc_w"][i], np.float32)
                 * ln2_g[i][None, :]).T
        mw[i] = np.asarray(inputs["mproj_w"][i], np.float32).T
    hwT = (np.asarray(inputs["head_w"], np.float32) * lnf_g[None, :]).T  # [C,V]

    common = dict(aw=_bf(aw), ala=_bf(ala), alb=_bf(alb), pw=_bf(pw),
                  pla=_bf(pla), plb=_bf(plb), fw=_bf(fw), mw=_bf(mw),
                  ident=_bf(np.eye(128, dtype=np.float32)))

    in_maps = []
    for c in range(NCORES):
        g, r = c // 4, c % 4
        sl = slice(r * TO, (r + 1) * TO)
        x0 = wte[idx[g, sl]] + wpe[sl]
        x0 = np.ascontiguousarray(x0.reshape(NTT, 128, C), np.float32)
        # masks[kc, kk, qq]: add 0 where key (kc*128+kk) <= query (r*TO+qq)
        kglob = (np.arange(T).reshape(8, 128))[:, :, None]
        qglob = r * TO + np.arange(TO)[None, None, :]
        masks = np.where(kglob <= qglob, 0.0, NEG).astype(np.float32)
        hw = np.zeros((C, VC), np.float32)
        lo, hi = c * VC, min((c + 1) * VC, V)
        hw[:, 0:hi - lo] = hwT[:, lo:hi]
        m = dict(common)
        m.update(x0=x0, masks=masks, hw=_bf(hw))
        in_maps.append(m)
    return in_maps


def kernel(**inputs):
    if "nc" not in _CACHE:
        _CACHE["nc"] = build_nc(debug=False)
    nc = _CACHE["nc"]
    in_maps = host_shards(inputs)
    res = bass_utils.run_bass_kernel_spmd(nc, in_maps,
                                          core_ids=list(range(NCORES)))
    out = np.empty((B * T, V), np.float32)
    for c in range(NCORES):
        lo, hi = c * VC, min((c + 1) * VC, V)
        out[:, lo:hi] = res.results[c]["y"][:, 0:hi - lo]
    return out.reshape(B, T, V)



# revision 6
# speedup vs baseline: 1.1788x; 1.1788x over previous
"""GPT (4-layer, C=512, H=8, T=1024, B=2, V=50257, LoRA r=8) on 8 trn2 cores.

Sharding: 2 groups of 4 cores (one per batch element); sequence-parallel
within a group (each core owns 256 tokens) with a per-layer KV AllGather;
vocab-sharded head matmul after a final all-core AllGather of x.
SPMD-uniform program: rank differences live in host-side data (causal masks,
padded head shards).
"""
import math
import numpy as np
import ml_dtypes

import concourse.bass as bass
import concourse.bacc as bacc
import concourse.tile as tile
import concourse.mybir as mybir
from concourse import bass_utils

BF16 = mybir.dt.bfloat16
F32 = mybir.dt.float32
AF = mybir.ActivationFunctionType

L, H, C, V, B, T = 4, 8, 512, 50257, 2, 1024
R = 8
NCORES = 8
TO = 256            # tokens owned per core
NTT = TO // 128     # 2 token tiles per core
NF = C // 128       # 4 feature tiles
HD = C // H         # 64 head dim
VC = 6283           # true vocab shard (8*6283 = 50264 >= 50257)
VCP = 6400          # padded shard: 50 slices of 128 (12.5 chunks of 512)
NEG = -1.0e9

_CACHE = {}


def build_nc(debug=False, do_layers=True, do_head=True, reps=1,
             no_coll=False, psum_dma=False):
    nc = bacc.Bacc("TRN2", target_bir_lowering=False, debug=False,
                   num_devices=NCORES)
    d = {}
    def inp(name, shape, dt):
        d[name] = nc.dram_tensor(name, shape, dt, kind="ExternalInput").ap()
    inp("x0", [NTT, 128, C], F32)
    inp("masks", [8, 128, TO], F32)
    inp("ident", [128, 128], BF16)
    inp("aw", [L, C, 3 * C], BF16)     # attn_w.T, q-cols pre-scaled
    inp("ala", [L, C, R], BF16)
    inp("alb", [L, R, 3 * C], BF16)    # *4.0, q-cols pre-scaled
    inp("pw", [L, C, C], BF16)
    inp("pla", [L, C, R], BF16)
    inp("plb", [L, R, C], BF16)        # *4.0
    inp("fw", [L, C, 4 * C], BF16)
    inp("mw", [L, 4 * C, C], BF16)
    inp("hw", [C, VCP], BF16)          # head shard (rank-dep, zero-padded)
    y_d = nc.dram_tensor("y", [VCP, 8 * TO], BF16, kind="ExternalOutput").ap()
    if debug:
        xdbg = nc.dram_tensor("xdbg", [L, NTT, 128, C], F32,
                              kind="ExternalOutput").ap()

    with tile.TileContext(nc) as tc:
        with (
            tc.tile_pool(name="persist", bufs=1) as pp,
            tc.tile_pool(name="wts", bufs=1) as wp,
            tc.tile_pool(name="acts", bufs=1) as ap_,
            tc.tile_pool(name="acts3", bufs=3) as ap3,
            tc.tile_pool(name="stats", bufs=3) as sp,
            tc.tile_pool(name="dram", bufs=2, space="DRAM") as dp,
            tc.tile_pool(name="psu", bufs=8, space="PSUM") as psu,
        ):
            ident = pp.tile([128, 128], BF16)
            nc.sync.dma_start(ident[:], d["ident"][:])
            zt = pp.tile([128, 1], F32)
            nc.vector.memset(zt[:], 0.0)
            eps = pp.tile([128, 1], F32)
            nc.vector.memset(eps[:], 1e-5)
            maskT = pp.tile([128, 8, TO], F32)
            nc.sync.dma_start(maskT[:], d["masks"].rearrange("k p q -> p k q"))

            x = [pp.tile([128, C], F32, name=f"x{tt}", tag=f"x{tt}") for tt in range(NTT)]

            kt_all = [pp.tile([128, T], BF16, name=f"kt{f}", tag=f"kt{f}") for f in range(NF)]
            v_aug = [pp.tile([128, H, HD + 1], BF16, name=f"va{kb}", tag=f"va{kb}")
                     for kb in range(T // 128)]
            for kb in range(T // 128):
                nc.vector.memset(v_aug[kb][:, :, HD:HD + 1], 1.0)

            def layernorm(src_tiles, eng_alt):
                """Return bf16 normalized tiles (gamma folded on host, beta==0)."""
                out = []
                for tt in range(NTT):
                    nm = sp.tile([128, 1], F32, name="nm", tag="nm")
                    nc.vector.reduce_sum(nm[:], src_tiles[tt][:],
                                         axis=mybir.AxisListType.X, negate=True)
                    nms = sp.tile([128, 1], F32, name="nms", tag="nms")
                    nc.vector.tensor_scalar_mul(nms[:], nm[:], 1.0 / C)
                    xc = ap_.tile([128, C], F32, name="xc", tag="xc")
                    nc.vector.tensor_scalar_add(xc[:], src_tiles[tt][:], nms[:])
                    sq = ap_.tile([128, C], BF16, name="sq", tag="sq")
                    ssq = sp.tile([128, 1], F32, name="ssq", tag="ssq")
                    nc.scalar.activation(sq[:], xc[:], AF.Square,
                                         bias=zt[:], accum_out=ssq[:])
                    std = sp.tile([128, 1], F32, name="std", tag="std")
                    nc.scalar.activation(std[:], ssq[:], AF.Sqrt,
                                         bias=eps[:], scale=1.0 / C)
                    rstd = sp.tile([128, 1], F32, name="rstd", tag="rstd")
                    nc.vector.reciprocal(rstd[:], std[:])
                    hb = ap_.tile([128, C], BF16, name=f"h{tt}", tag=f"h{tt}")
                    nc.vector.tensor_scalar_mul(hb[:], xc[:], rstd[:])
                    out.append(hb)
                return out

            def transpose_128(src_ap, dst_ap, eng):
                ptr = psu.tile([128, 128], BF16, name="tr", tag="u")
                nc.tensor.transpose(ptr[:], src_ap, ident[:])
                if eng == 0:
                    nc.scalar.copy(dst_ap, ptr[:])
                else:
                    nc.vector.tensor_copy(dst_ap, ptr[:])

            def transpose_tiles(tiles, nfree, tag):
                """tiles: list of [128, nfree*128] (token-major) ->
                list of nfree tiles [128, len(tiles)*128] (feature-major)."""
                outs = [ap_.tile([128, len(tiles) * 128], BF16, name=f"{tag}{f}", tag=f"{tag}{f}")
                        for f in range(nfree)]
                e = 0
                for i, t in enumerate(tiles):
                    for f in range(nfree):
                        transpose_128(t[:, f * 128:(f + 1) * 128],
                                      outs[f][:, i * 128:(i + 1) * 128], e % 2)
                        e += 1
                return outs

            for _rep in range(reps):
                for tt in range(NTT):
                    nc.sync.dma_start(x[tt][:], d["x0"][tt])
                for li in range(L if do_layers else 0):
                    aw = wp.tile([128, NF, 3 * C], BF16, name="aw", tag="aw", bufs=2)
                    nc.sync.dma_start(aw[:], d["aw"][li].rearrange(
                        "(f p) n -> p f n", p=128))
                    ala = wp.tile([128, NF, R], BF16, name="ala", tag="ala")
                    nc.sync.dma_start(ala[:], d["ala"][li].rearrange(
                        "(f p) n -> p f n", p=128))
                    alb = wp.tile([R, 3 * C], BF16, name="alb", tag="alb")
                    nc.sync.dma_start(alb[:], d["alb"][li])
                    pw = wp.tile([128, NF, C], BF16, name="pw", tag="pw")
                    nc.sync.dma_start(pw[:], d["pw"][li].rearrange(
                        "(f p) n -> p f n", p=128))
                    pla = wp.tile([128, NF, R], BF16, name="pla", tag="pla")
                    nc.sync.dma_start(pla[:], d["pla"][li].rearrange(
                        "(f p) n -> p f n", p=128))
                    plb = wp.tile([R, C], BF16, name="plb", tag="plb")
                    nc.sync.dma_start(plb[:], d["plb"][li])
                    fw = wp.tile([128, NF, 4 * C], BF16, name="fw", tag="fw", bufs=2)
                    nc.sync.dma_start(fw[:], d["fw"][li].rearrange(
                        "(f p) n -> p f n", p=128))
                    mw = wp.tile([128, 16, C], BF16, name="mw", tag="mw", bufs=2)
                    nc.sync.dma_start(mw[:], d["mw"][li].rearrange(
                        "(f p) n -> p f n", p=128))

                    # ---- attention ----
                    h = layernorm(x, 0)
                    hT = transpose_tiles(h, NF, "hT")

                    # LoRA down-proj: z [R, TO]
                    pz = psu.tile([R, TO], F32, name="z", tag="u")
                    for f in range(NF):
                        nc.tensor.matmul(pz[:], ala[:, f, :], hT[f][:],
                                         start=(f == 0), stop=(f == NF - 1))
                    z = sp.tile([R, TO], BF16, name="z", tag="z")
                    nc.scalar.copy(z[:], pz[:])

                    # qkvT [3C, TO] feature-major, 12 chunks of 128
                    qT = [ap_.tile([128, TO], BF16, name=f"qT{f}", tag=f"qT{f}") for f in range(NF)]
                    vT = [ap_.tile([128, TO], BF16, name=f"vT{f}", tag=f"vT{f}") for f in range(NF)]
                    kt_own = ap_.tile([128, NF, TO], BF16, name="kt_own", tag="kt_own")
                    for ch in range(3 * NF):
                        pq = psu.tile([128, TO], F32, name="mm256", tag="u")
                        for f in range(NF):
                            nc.tensor.matmul(pq[:], aw[:, f, ch * 128:(ch + 1) * 128],
                                             hT[f][:], start=(f == 0), stop=False)
                        nc.tensor.matmul(pq[:], alb[:, ch * 128:(ch + 1) * 128],
                                         z[:], start=False, stop=True)
                        if ch < NF:
                            nc.scalar.copy(qT[ch][:], pq[:])
                        elif ch < 2 * NF:
                            nc.scalar.copy(kt_own[:, ch - NF, :], pq[:])
                        else:
                            nc.vector.tensor_copy(vT[ch - 2 * NF][:], pq[:])

                    # V own -> token-major [TO, C] stored flat [128, (t c)]
                    v_own = ap_.tile([128, NTT * C], BF16, name="v_own",
                                     tag="v_own")
                    e = 0
                    for f in range(NF):
                        for t in range(NTT):
                            transpose_128(vT[f][:, t * 128:(t + 1) * 128],
                                          v_own[:, t * C + f * 128:
                                                t * C + (f + 1) * 128], e % 2)
                            e += 1

                    # KV AllGather within group of 4
                    cin = dp.tile([2, 128, NF, TO], BF16, name="cin", tag="cin")
                    cout = dp.tile([4, 2, 128, NF, TO], BF16, name="cout",
                                   tag="cout")
                    nc.sync.dma_start(cin[0], kt_own[:])
                    nc.sync.dma_start(
                        cin[1], v_own[:].rearrange("p (f q) -> p f q", f=NF))
                    nc.gpsimd.collective_compute(
                        "AllGather", mybir.AluOpType.bypass,
                        ins=[cin.opt()], outs=[cout.opt()],
                        replica_groups=[[0, 1, 2, 3], [4, 5, 6, 7]],
                    )
                    for r in range(4):
                        for f in range(NF):
                            nc.sync.dma_start(kt_all[f][:, r * TO:(r + 1) * TO],
                                              cout[r, 0, :, f, :])
                        for t in range(NTT):
                            kb = 2 * r + t
                            nc.sync.dma_start(
                                v_aug[kb][:, :, 0:HD],
                                cout[r, 1, :, 2 * t:2 * t + 2, :].rearrange(
                                    "p a (x e) -> p (a x) e", x=NF))

                    # attention: ST [k, q] per head per k-chunk; P=exp; AV
                    y_sb = [ap_.tile([128, C], BF16, name=f"y{tt}", tag=f"y{tt}")
                            for tt in range(NTT)]
                    pyav = [[None] * 2 for _ in range(NTT)]
                    for hh in range(H):
                        f, po = hh // 2, (hh % 2) * HD
                        pts = []
                        for kp in range(T // 256):
                            pst = psu.tile([128, 2, TO], F32, name="mm256",
                                           tag="u")
                            for j in range(2):
                                kc = 2 * kp + j
                                nc.tensor.matmul(
                                    pst[:, j, :],
                                    kt_all[f][po:po + HD,
                                              kc * 128:(kc + 1) * 128],
                                    qT[f][po:po + HD, :],
                                    start=True, stop=True)
                            nc.vector.tensor_add(
                                pst[:], pst[:],
                                maskT[:, 2 * kp:2 * kp + 2, :])
                            pt = ap3.tile([128, 2, TO], BF16, name="pt",
                                          tag="pt", bufs=4)
                            nc.scalar.activation(pt[:], pst[:], AF.Exp,
                                                 bias=zt[:])
                            pts.append(pt)
                        for tt in range(NTT):
                            hb, hi = hh // 4, hh % 4
                            if hi == 0:
                                pyav[tt][hb] = psu.tile([128, 4, HD + 1], F32,
                                                          name="yav", tag="u")
                            for kc in range(T // 128):
                                nc.tensor.matmul(
                                    pyav[tt][hb][:, hi, :],
                                    pts[kc // 2][:, kc % 2,
                                                 tt * 128:(tt + 1) * 128],
                                    v_aug[kc][:, hh, :],
                                    start=(kc == 0), stop=(kc == T // 128 - 1))
                            if hi == 3:
                                sums = sp.tile([128, 4], F32, name="sums", tag="sums")
                                for j in range(4):
                                    nc.scalar.copy(sums[:, j:j + 1],
                                                   pyav[tt][hb][:, j, HD:HD + 1])
                                rec = sp.tile([128, 4], F32, name="rec", tag="rec")
                                nc.vector.reciprocal(rec[:], sums[:])
                                for j in range(4):
                                    hj = hb * 4 + j
                                    nc.vector.tensor_scalar_mul(
                                        y_sb[tt][:, hj * HD:(hj + 1) * HD],
                                        pyav[tt][hb][:, j, 0:HD],
                                        rec[:, j:j + 1])

                    # proj + LoRA + residual
                    yT = transpose_tiles(y_sb, NF, "yT")
                    pz2 = psu.tile([R, TO], F32, name="z", tag="u")
                    for f in range(NF):
                        nc.tensor.matmul(pz2[:], pla[:, f, :], yT[f][:],
                                         start=(f == 0), stop=(f == NF - 1))
                    z2 = sp.tile([R, TO], BF16, name="z", tag="z")
                    nc.scalar.copy(z2[:], pz2[:])
                    for tt in range(NTT):
                        pp_ = psu.tile([128, C], F32, name="mm512", tag="u")
                        for f in range(NF):
                            nc.tensor.matmul(pp_[:], yT[f][:, tt * 128:(tt + 1) * 128],
                                             pw[:, f, :], start=(f == 0), stop=False)
                        nc.tensor.matmul(pp_[:], z2[:, tt * 128:(tt + 1) * 128],
                                         plb[:], start=False, stop=True)
                        nc.vector.tensor_add(x[tt][:], x[tt][:], pp_[:])

                    # ---- MLP ----
                    h2 = layernorm(x, 1)
                    h2T = transpose_tiles(h2, NF, "h2T")
                    # fc output computed feature-major: mF[g] = [128, TO],
                    # g-th 128-slice of the 2048 hidden dim; no transposes.
                    mF = [ap_.tile([128, TO], BF16, name=f"mF{g}", tag=f"mF{g}")
                          for g in range(16)]
                    for gp in range(8):
                        pf = psu.tile([128, 2, TO], F32, name="mm512", tag="u")
                        for j in range(2):
                            g = 2 * gp + j
                            for f in range(NF):
                                nc.tensor.matmul(
                                    pf[:, j, :],
                                    fw[:, f, g * 128:(g + 1) * 128],
                                    h2T[f][:],
                                    start=(f == 0), stop=(f == NF - 1))
                        nc.scalar.activation(mF[2 * gp][:], pf[:, 0, :],
                                             AF.Gelu_apprx_tanh, bias=zt[:])
                        nc.scalar.activation(mF[2 * gp + 1][:], pf[:, 1, :],
                                             AF.Gelu_apprx_tanh, bias=zt[:])
                    for tt in range(NTT):
                        pm = psu.tile([128, C], F32, name="mm512", tag="u")
                        for f in range(16):
                            nc.tensor.matmul(pm[:], mF[f][:, tt * 128:(tt + 1) * 128],
                                             mw[:, f, :],
                                             start=(f == 0), stop=(f == 15))
                        nc.vector.tensor_add(x[tt][:], x[tt][:], pm[:])
                        if debug:
                            nc.sync.dma_start(xdbg[li, tt], x[tt][:])

                if do_head:
                    # ---- final LN + all-core AllGather of xT + head ----
                    xf = layernorm(x, 0)
                    xfT = transpose_tiles(xf, NF, "xfT")
                    cinF = dp.tile([NF, 128, TO], BF16, name="cinF", tag="cinF")
                    coutF = dp.tile([8, NF, 128, TO], BF16, name="coutF", tag="coutF",
                                     addr_space="Shared")
                    for f in range(NF):
                        nc.sync.dma_start(cinF[f], xfT[f][:])
                    nc.gpsimd.collective_compute(
                        "AllGather", mybir.AluOpType.bypass,
                        ins=[cinF.opt()], outs=[coutF.opt()],
                        replica_groups=[[0, 1, 2, 3, 4, 5, 6, 7]],
                    )
                    xT_all = [pp.tile([128, 8 * TO], BF16, name=f"xta{f}", tag=f"xta{f}")
                              for f in range(NF)]
                    for r in range(8):
                        for f in range(NF):
                            nc.sync.dma_start(xT_all[f][:, r * TO:(r + 1) * TO],
                                              coutF[r, f])
                    # head: out partition = vocab slice (128), free = tokens.
                    # Per 128-voc slice: 4 token groups x 4 f-accum matmuls
                    # (512-wide), copied f32->bf16 into a [128, 2048] staging
                    # tile, then ONE contiguous 512KB DMA per slice.
                    for ch in range(13):
                        nch = min(512, VCP - ch * 512)
                        hwt = wp.tile([128, NF, 512], BF16, name="hw", tag="hw", bufs=2)
                        nc.sync.dma_start(
                            hwt[:, :, 0:nch],
                            d["hw"][:, ch * 512:ch * 512 + nch].rearrange(
                                "(f p) n -> p f n", p=128))
                        for v in range(nch // 128):
                            stage = ap3.tile([128, 8 * TO], BF16, name="lo",
                                             tag="lo", bufs=3)
                            for tg in range(4):
                                pl = psu.tile([128, 512], F32, name="mm512",
                                              tag="u")
                                for f in range(NF):
                                    nc.tensor.matmul(
                                        pl[:],
                                        hwt[:, f, v * 128:(v + 1) * 128],
                                        xT_all[f][:, tg * 512:(tg + 1) * 512],
                                        start=(f == 0), stop=(f == NF - 1))
                                if tg % 2 == 0:
                                    nc.scalar.copy(
                                        stage[:, tg * 512:(tg + 1) * 512], pl[:])
                                else:
                                    nc.vector.tensor_copy(
                                        stage[:, tg * 512:(tg + 1) * 512], pl[:])
                            vg = ch * 4 + v
                            nc.sync.dma_start(
                                y_d[vg * 128:(vg + 1) * 128, :], stage[:])

    nc.compile()
    return nc


def _bf(a):
    return np.ascontiguousarray(a.astype(ml_dtypes.bfloat16))


def host_shards(inputs, debug=False):
    idx = np.asarray(inputs["idx"])
    wte = np.asarray(inputs["wte"], np.float32)
    wpe = np.asarray(inputs["wpe"], np.float32)
    ln1_g = np.asarray(inputs["ln1_g"], np.float32)
    ln2_g = np.asarray(inputs["ln2_g"], np.float32)
    lnf_g = np.asarray(inputs["lnf_g"], np.float32)
    for nm in ("ln1_b", "ln2_b", "fc_b", "mproj_b", "lnf_b"):
        assert np.abs(np.asarray(inputs[nm])).max() == 0.0, f"{nm} nonzero"
    LS = 32.0 / 8.0
    qs = 1.0 / math.sqrt(HD)

    aw = np.empty((L, C, 3 * C), np.float32)
    ala = np.empty((L, C, R), np.float32)
    alb = np.empty((L, R, 3 * C), np.float32)
    pw = np.empty((L, C, C), np.float32)
    pla = np.empty((L, C, R), np.float32)
    plb = np.empty((L, R, C), np.float32)
    fw = np.empty((L, C, 4 * C), np.float32)
    mw = np.empty((L, 4 * C, C), np.float32)
    for i in range(L):
        a = (np.asarray(inputs["attn_w"][i], np.float32) * ln1_g[i][None, :]).T
        a = a.copy()
        a[:, :C] *= qs
        aw[i] = a
        ala[i] = (np.asarray(inputs["attn_lA"][i], np.float32)
                  * ln1_g[i][None, :]).T
        b = np.asarray(inputs["attn_lB"][i], np.float32).T * LS
        b = b.copy()
        b[:, :C] *= qs
        alb[i] = b
        pw[i] = np.asarray(inputs["proj_w"][i], np.float32).T
        pla[i] = np.asarray(inputs["proj_lA"][i], np.float32).T
        plb[i] = np.asarray(inputs["proj_lB"][i], np.float32).T * LS
        fw[i] = (np.asarray(inputs["fc_w"][i], np.float32)
                 * ln2_g[i][None, :]).T
        mw[i] = np.asarray(inputs["mproj_w"][i], np.float32).T
    hwT = (np.asarray(inputs["head_w"], np.float32) * lnf_g[None, :]).T  # [C,V]

    common = dict(aw=_bf(aw), ala=_bf(ala), alb=_bf(alb), pw=_bf(pw),
                  pla=_bf(pla), plb=_bf(plb), fw=_bf(fw), mw=_bf(mw),
                  ident=_bf(np.eye(128, dtype=np.float32)))

    in_maps = []
    for c in range(NCORES):
        g, r = c // 4, c % 4
        sl = slice(r * TO, (r + 1) * TO)
        x0 = wte[idx[g, sl]] + wpe[sl]
        x0 = np.ascontiguousarray(x0.reshape(NTT, 128, C), np.float32)
        # masks[kc, kk, qq]: add 0 where key (kc*128+kk) <= query (r*TO+qq)
        kglob = (np.arange(T).reshape(8, 128))[:, :, None]
        qglob = r * TO + np.arange(TO)[None, None, :]
        masks = np.where(kglob <= qglob, 0.0, NEG).astype(np.float32)
        hw = np.zeros((C, VCP), np.float32)
        lo, hi = c * VC, min((c + 1) * VC, V)
        hw[:, 0:hi - lo] = hwT[:, lo:hi]
        m = dict(common)
        m.update(x0=x0, masks=masks, hw=_bf(hw))
        in_maps.append(m)
    return in_maps


def kernel(**inputs):
    if "nc" not in _CACHE:
        _CACHE["nc"] = build_nc(debug=False)
    nc = _CACHE["nc"]
    in_maps = host_shards(inputs)
    res = bass_utils.run_bass_kernel_spmd(nc, in_maps,
                                          core_ids=list(range(NCORES)))
    out = np.empty((B * T, V), np.float32)
    for c in range(NCORES):
        lo, hi = c * VC, min((c + 1) * VC, V)
        out[:, lo:hi] = res.results[c]["y"][0:hi - lo, :].T.astype(np.float32)
    return out.reshape(B, T, V)



# revision 26
# speedup vs baseline: 1.2703x; 1.0776x over previous
"""GPT (4-layer, C=512, H=8, T=1024, B=2, V=50257, LoRA r=8) on 8 trn2 cores.

Sharding: every core owns global token tile c (128 tokens) of BOTH batches.
The two batch streams are software-pipelined inside each layer so the
per-batch 8-rank KV AllGather latency hides under the other batch's compute:
    qkv(i,b0) -> fire AG0 ; mlp(i-1,b1) ; qkv(i,b1) -> fire AG1 ;
    attn(i,b0) ; attn(i,b1) ; mlp(i,b0) ; [mlp(i,b1) deferred]
Head: vocab-sharded (6400-padded shard per core), out partition = vocab,
bf16 logits, 50 contiguous 512KB output DMAs; host upcasts/transposes.
"""
import math
import numpy as np
import ml_dtypes

import concourse.bass as bass
import concourse.bacc as bacc
import concourse.tile as tile
import concourse.mybir as mybir
from concourse import bass_utils

BF16 = mybir.dt.bfloat16
F32 = mybir.dt.float32
AF = mybir.ActivationFunctionType

L, H, C, V, B, T = 4, 8, 512, 50257, 2, 1024
R = 8
NCORES = 8
NF = C // 128        # 4 feature tiles
HD = C // H          # 64 head dim
VC = 6283            # true vocab shard (8*6283 = 50264 >= 50257)
VCP = 6400           # padded shard: 50 slices of 128
NEG = -1.0e9

_CACHE = {}


def build_nc(debug=False):
    nc = bacc.Bacc("TRN2", target_bir_lowering=False, debug=False,
                   num_devices=NCORES)
    d = {}
    def inp(name, shape, dt):
        d[name] = nc.dram_tensor(name, shape, dt, kind="ExternalInput").ap()
    inp("x0", [2, 128, C], F32)        # [batch, own 128 tokens, C]
    inp("maskT", [128, 8, 128], BF16)  # [kk, kc, qq] causal add-mask
    inp("ident", [128, 128], BF16)
    inp("aw", [L, C, 3 * C], BF16)     # attn_w.T, q-cols pre-scaled
    inp("ala", [L, C, R], BF16)
    inp("alb", [L, R, 3 * C], BF16)    # *4.0, q-cols pre-scaled
    inp("pw", [L, C, C], BF16)
    inp("pla", [L, C, R], BF16)
    inp("plb", [L, R, C], BF16)        # *4.0
    inp("fw", [L, C, 4 * C], BF16)
    inp("mw", [L, 4 * C, C], BF16)
    inp("hw", [C, VCP], BF16)          # head shard (rank-dep, zero-padded)
    y_d = nc.dram_tensor("y", [VCP, 2 * T], BF16, kind="ExternalOutput").ap()
    if debug:
        xdbg = nc.dram_tensor("xdbg", [L, 2, 128, C], F32,
                              kind="ExternalOutput").ap()

    with tile.TileContext(nc) as tc:
        with (
            tc.tile_pool(name="persist", bufs=1) as pp,
            tc.tile_pool(name="wts", bufs=1) as wp,
            tc.tile_pool(name="acts", bufs=1) as ap_,
            tc.tile_pool(name="acts3", bufs=3) as ap3,
            tc.tile_pool(name="stats", bufs=3) as sp,
            tc.tile_pool(name="dram", bufs=2, space="DRAM") as dp,
            tc.tile_pool(name="psu", bufs=8, space="PSUM") as psu,
        ):
            ident = pp.tile([128, 128], BF16)
            nc.sync.dma_start(ident[:], d["ident"][:])
            zt = pp.tile([128, 1], F32)
            nc.vector.memset(zt[:], 0.0)
            eps = pp.tile([128, 1], F32)
            nc.vector.memset(eps[:], 1e-5)
            maskT = pp.tile([128, 8, 128], BF16)
            nc.sync.dma_start(maskT[:], d["maskT"][:])

            x = [pp.tile([128, C], F32, name=f"x{b}", tag=f"x{b}")
                 for b in range(2)]
            kt_all = [[pp.tile([128, T], BF16, name=f"kt{b}{f}", tag=f"kt{b}{f}")
                       for f in range(NF)] for b in range(2)]
            v_aug = [pp.tile([128, 8, H, HD + 1], BF16, name=f"va{b}",
                             tag=f"va{b}") for b in range(2)]
            for b in range(2):
                nc.vector.memset(v_aug[b][:, :, :, HD:HD + 1], 1.0)

            def layernorm(xt, tag):
                """One token tile [128, C] f32 -> bf16 normalized."""
                nm = sp.tile([128, 1], F32, name="nm", tag="nm")
                nc.vector.reduce_sum(nm[:], xt[:],
                                     axis=mybir.AxisListType.X, negate=True)
                nms = sp.tile([128, 1], F32, name="nms", tag="nms")
                nc.vector.tensor_scalar_mul(nms[:], nm[:], 1.0 / C)
                xc = ap_.tile([128, C], F32, name="xc", tag="xc", bufs=2)
                nc.vector.tensor_scalar_add(xc[:], xt[:], nms[:])
                sq = ap_.tile([128, C], BF16, name="sq", tag="sq", bufs=1)
                ssq = sp.tile([128, 1], F32, name="ssq", tag="ssq")
                nc.scalar.activation(sq[:], xc[:], AF.Square,
                                     bias=zt[:], accum_out=ssq[:])
                std = sp.tile([128, 1], F32, name="std", tag="std")
                nc.scalar.activation(std[:], ssq[:], AF.Sqrt,
                                     bias=eps[:], scale=1.0 / C)
                rstd = sp.tile([128, 1], F32, name="rstd", tag="rstd")
                nc.vector.reciprocal(rstd[:], std[:])
                hb = ap_.tile([128, C], BF16, name=f"h{tag}", tag=f"h{tag}")
                nc.vector.tensor_scalar_mul(hb[:], xc[:], rstd[:])
                return hb

            def transpose_128(src_ap, dst_ap, eng):
                ptr = psu.tile([128, 128], BF16, name="tr", tag="u")
                nc.tensor.transpose(ptr[:], src_ap, ident[:])
                if eng == 0:
                    nc.scalar.copy(dst_ap, ptr[:])
                else:
                    nc.vector.tensor_copy(dst_ap, ptr[:])

            def transpose_feat(h_b, tag):
                """h_b [128 tok, C] -> list of NF tiles [128 f, 128 tok]."""
                outs = []
                for f in range(NF):
                    t = ap_.tile([128, 128], BF16, name=f"{tag}{f}",
                                 tag=f"{tag}{f}", bufs=1)
                    transpose_128(h_b[:, f * 128:(f + 1) * 128], t[:], f % 2)
                    outs.append(t)
                return outs

            # weight tiles, loaded per layer (double-buffered)
            def load_weights(li):
                w = {}
                w["aw"] = wp.tile([128, NF, 3 * C], BF16, name="aw", tag="aw",
                                  bufs=2)
                nc.sync.dma_start(w["aw"][:], d["aw"][li].rearrange(
                    "(f p) n -> p f n", p=128))
                w["ala"] = wp.tile([128, NF, R], BF16, name="ala", tag="ala",
                                   bufs=2)
                nc.sync.dma_start(w["ala"][:], d["ala"][li].rearrange(
                    "(f p) n -> p f n", p=128))
                w["alb"] = wp.tile([R, 3 * C], BF16, name="alb", tag="alb",
                                   bufs=2)
                nc.sync.dma_start(w["alb"][:], d["alb"][li])
                w["pw"] = wp.tile([128, NF, C], BF16, name="pw", tag="pw",
                                  bufs=2)
                nc.sync.dma_start(w["pw"][:], d["pw"][li].rearrange(
                    "(f p) n -> p f n", p=128))
                w["pla"] = wp.tile([128, NF, R], BF16, name="pla", tag="pla",
                                   bufs=2)
                nc.sync.dma_start(w["pla"][:], d["pla"][li].rearrange(
                    "(f p) n -> p f n", p=128))
                w["plb"] = wp.tile([R, C], BF16, name="plb", tag="plb",
                                   bufs=2)
                nc.sync.dma_start(w["plb"][:], d["plb"][li])
                w["fw"] = wp.tile([128, NF, 4 * C], BF16, name="fw", tag="fw",
                                  bufs=2)
                nc.sync.dma_start(w["fw"][:], d["fw"][li].rearrange(
                    "(f p) n -> p f n", p=128))
                w["mw"] = wp.tile([128, 16, C], BF16, name="mw", tag="mw",
                                  bufs=2)
                nc.sync.dma_start(w["mw"][:], d["mw"][li].rearrange(
                    "(f p) n -> p f n", p=128))
                return w

            # per-(layer,batch) attention state
            def qkv_block(w, b):
                """LN1, transposes, qkv matmul, fire the KV AllGather."""
                st = {}
                h = layernorm(x[b], f"1b{b}")
                hT = transpose_feat(h, f"hT{b}")
                pz = psu.tile([R, 128], F32, name="z", tag="u")
                for f in range(NF):
                    nc.tensor.matmul(pz[:], w["ala"][:, f, :], hT[f][:],
                                     start=(f == 0), stop=(f == NF - 1))
                z = sp.tile([R, 128], BF16, name="z", tag="z")
                nc.scalar.copy(z[:], pz[:])

                qT = ap_.tile([128, NF, 128], BF16, name=f"qT{b}",
                              tag=f"qT{b}")
                ktm = ap_.tile([128, NF, 128], BF16, name=f"ktm{b}",
                               tag=f"ktm{b}")
                vT = [ap_.tile([128, 128], BF16, name=f"vT{b}{f}",
                               tag=f"vT{b}{f}") for f in range(NF)]
                for chg in range(3):
                    pq = psu.tile([128, NF, 128], F32, name="mmq", tag="u")
                    for c4 in range(NF):
                        ch = chg * NF + c4
                        for f in range(NF):
                            nc.tensor.matmul(
                                pq[:, c4, :],
                                w["aw"][:, f, ch * 128:(ch + 1) * 128],
                                hT[f][:], start=(f == 0), stop=False)
                        nc.tensor.matmul(
                            pq[:, c4, :],
                            w["alb"][:, ch * 128:(ch + 1) * 128],
                            z[:], start=False, stop=True)
                    if chg == 0:
                        nc.scalar.copy(qT[:], pq[:])
                    elif chg == 1:
                        nc.vector.tensor_copy(ktm[:], pq[:])
                    else:
                        for f in range(NF):
                            if f % 2 == 0:
                                nc.scalar.copy(vT[f][:], pq[:, f, :])
                            else:
                                nc.vector.tensor_copy(vT[f][:], pq[:, f, :])
                # v token-major [128 tok, C]
                v_own = ap_.tile([128, C], BF16, name=f"vo{b}", tag=f"vo{b}")
                for f in range(NF):
                    transpose_128(vT[f][:], v_own[:, f * 128:(f + 1) * 128],
                                  (f + 1) % 2)
                cin = dp.tile([2, 128, NF, 128], BF16, name=f"cin{b}",
                              tag=f"cin{b}")
                cout = dp.tile([8, 2, 128, NF, 128], BF16, name=f"cout{b}",
                               tag=f"cout{b}", addr_space="Shared")
                nc.sync.dma_start(cin[0], ktm[:])
                nc.sync.dma_start(
                    cin[1], v_own[:].rearrange("p (f q) -> p f q", f=NF))
                nc.gpsimd.collective_compute(
                    "AllGather", mybir.AluOpType.bypass,
                    ins=[cin.opt()], outs=[cout.opt()],
                    replica_groups=[[0, 1, 2, 3, 4, 5, 6, 7]],
                )
                st["qT"] = qT
                st["cout"] = cout
                return st

            def attn_block(w, b, st):
                """Scatter AG result, scores+softmax+AV, proj, residual."""
                qT, cout = st["qT"], st["cout"]
                for f in range(NF):
                    nc.sync.dma_start(
                        kt_all[b][f][:].rearrange("p (r k) -> p r k", r=8),
                        cout[:, 0, :, f, :].rearrange("r p k -> p r k"))
                for r in range(8):
                    nc.sync.dma_start(
                        v_aug[b][:, r, :, 0:HD],
                        cout[r, 1, :, :, :].rearrange(
                            "p f (g e) -> p (f g) e", g=2))

                y_sb = ap_.tile([128, C], BF16, name=f"y{b}", tag=f"y{b}")
                pts = {}
                for hh in range(H):
                    f, po = hh // 2, (hh % 2) * HD
                    for kpg in range(2):
                        pst = psu.tile([128, 4, 128], F32, name="mms",
                                       tag="u")
                        for j in range(4):
                            kc = kpg * 4 + j
                            nc.tensor.matmul(
                                pst[:, j, :],
                                kt_all[b][f][po:po + HD,
                                             kc * 128:(kc + 1) * 128],
                                qT[po:po + HD, f, :],
                                start=True, stop=True)
                        nc.vector.tensor_add(
                            pst[:], pst[:], maskT[:, kpg * 4:kpg * 4 + 4, :])
                        pt = ap3.tile([128, 4, 128], BF16, name="pt",
                                      tag="pt", bufs=9)
                        nc.scalar.activation(pt[:], pst[:], AF.Exp,
                                             bias=zt[:])
                        pts[(hh, kpg)] = pt
                    if hh % 4 == 3:
                        hb = hh // 4
                        pyav = psu.tile([128, 4, HD + 1], F32, name="yav",
                                        tag="u")
                        for hi in range(4):
                            h2 = hb * 4 + hi
                            for kb in range(8):
                                nc.tensor.matmul(
                                    pyav[:, hi, :],
                                    pts[(h2, kb // 4)][:, kb % 4, :],
                                    v_aug[b][:, kb, h2, :],
                                    start=(kb == 0), stop=(kb == 7))
                        sums = sp.tile([128, 4], F32, name="sums", tag="sums")
                        for j in range(4):
                            nc.scalar.copy(sums[:, j:j + 1],
                                           pyav[:, j, HD:HD + 1])
                        rec = sp.tile([128, 4], F32, name="rec", tag="rec")
                        nc.vector.reciprocal(rec[:], sums[:])
                        for j in range(4):
                            hj = hb * 4 + j
                            nc.vector.tensor_scalar_mul(
                                y_sb[:, hj * HD:(hj + 1) * HD],
                                pyav[:, j, 0:HD], rec[:, j:j + 1])

                # proj + LoRA + residual
                yT = transpose_feat(y_sb, f"yT{b}")
                pz2 = psu.tile([R, 128], F32, name="z", tag="u")
                for f in range(NF):
                    nc.tensor.matmul(pz2[:], w["pla"][:, f, :], yT[f][:],
                                     start=(f == 0), stop=(f == NF - 1))
                z2 = sp.tile([R, 128], BF16, name="z2", tag="z2")
                nc.scalar.copy(z2[:], pz2[:])
                pp_ = psu.tile([128, C], F32, name="mm512", tag="u")
                for f in range(NF):
                    nc.tensor.matmul(pp_[:], yT[f][:], w["pw"][:, f, :],
                                     start=(f == 0), stop=False)
                nc.tensor.matmul(pp_[:], z2[:], w["plb"][:],
                                 start=False, stop=True)
                nc.vector.tensor_add(x[b][:], x[b][:], pp_[:])

            def mlp_block(w, b, li):
                h2 = layernorm(x[b], f"2b{b}")
                h2T = transpose_feat(h2, f"h2T{b}")
                mF = ap_.tile([128, 16, 128], BF16, name=f"mF{b}",
                              tag=f"mF{b}")
                for gq in range(4):
                    pf = psu.tile([128, 4, 128], F32, name="mmf", tag="u")
                    for gi in range(4):
                        g = gq * 4 + gi
                        for f in range(NF):
                            nc.tensor.matmul(
                                pf[:, gi, :],
                                w["fw"][:, f, g * 128:(g + 1) * 128],
                                h2T[f][:],
                                start=(f == 0), stop=(f == NF - 1))
                    nc.scalar.activation(mF[:, gq * 4:gq * 4 + 4, :], pf[:],
                                         AF.Gelu_apprx_tanh, bias=zt[:])
                pm = psu.tile([128, C], F32, name="mm512", tag="u")
                for g in range(16):
                    nc.tensor.matmul(pm[:], mF[:, g, :], w["mw"][:, g, :],
                                     start=(g == 0), stop=(g == 15))
                nc.vector.tensor_add(x[b][:], x[b][:], pm[:])
                if debug:
                    nc.sync.dma_start(xdbg[li, b], x[b][:])

            # ---- prologue ----
            for b in range(2):
                nc.sync.dma_start(x[b][:], d["x0"][b])

            # ---- pipelined layers ----
            wts = [None, None]
            wts[0] = load_weights(0)
            st = [None, None]
            for li in range(L):
                w = wts[li % 2]
                st[0] = qkv_block(w, 0)
                if li > 0:
                    # deferred MLP of the other batch hides AG0's latency;
                    # it is the LAST reader of layer li-1's weights, so the
                    # li+1 prefetch (same slots) must be emitted after it.
                    mlp_block(wts[(li - 1) % 2], 1, li - 1)
                if li + 1 < L:
                    wts[(li + 1) % 2] = load_weights(li + 1)
                st[1] = qkv_block(w, 1)
                attn_block(w, 0, st[0])
                attn_block(w, 1, st[1])
                mlp_block(w, 0, li)
            mlp_block(wts[(L - 1) % 2], 1, L - 1)

            # ---- final LN + 8-rank AllGather of xfT + head ----
            xf = [layernorm(x[b], f"fb{b}") for b in range(2)]
            xfT = [ap_.tile([128, 256], BF16, name=f"xfT{f}", tag=f"xfT{f}")
                   for f in range(NF)]
            e = 0
            for b in range(2):
                for f in range(NF):
                    transpose_128(xf[b][:, f * 128:(f + 1) * 128],
                                  xfT[f][:, b * 128:(b + 1) * 128], e % 2)
                    e += 1
            cinF = dp.tile([NF, 128, 256], BF16, name="cinF", tag="cinF")
            coutF = dp.tile([8, NF, 128, 256], BF16, name="coutF",
                            tag="coutF", addr_space="Shared")
            for f in range(NF):
                nc.sync.dma_start(cinF[f], xfT[f][:])
            nc.gpsimd.collective_compute(
                "AllGather", mybir.AluOpType.bypass,
                ins=[cinF.opt()], outs=[coutF.opt()],
                replica_groups=[[0, 1, 2, 3, 4, 5, 6, 7]],
            )
            # reuse kt_all tiles (dead after layers) as gathered-xfT storage:
            # batch b tokens live in kt_all[b][f] [128, 1024]
            for f in range(NF):
                for b in range(2):
                    nc.sync.dma_start(
                        kt_all[b][f][:].rearrange("p (r q) -> p r q", r=8),
                        coutF[:, f, :, b * 128:(b + 1) * 128].rearrange(
                            "r p q -> p r q"))

            # head: out partition = vocab slice (128), free = tokens.
            for ch in range(13):
                nch = min(512, VCP - ch * 512)
                hwt = wp.tile([128, NF, 512], BF16, name="hw", tag="hw",
                              bufs=2)
                nc.sync.dma_start(
                    hwt[:, :, 0:nch],
                    d["hw"][:, ch * 512:ch * 512 + nch].rearrange(
                        "(f p) n -> p f n", p=128))
                for v in range(nch // 128):
                    stage = ap3.tile([128, 2 * T], BF16, name="lo",
                                     tag="lo", bufs=2)
                    for tg in range(4):
                        pl = psu.tile([128, 512], F32, name="mm512", tag="u")
                        for f in range(NF):
                            nc.tensor.matmul(
                                pl[:],
                                hwt[:, f, v * 128:(v + 1) * 128],
                                kt_all[tg // 2][f][:, (tg % 2) * 512:
                                                   (tg % 2) * 512 + 512],
                                start=(f == 0), stop=(f == NF - 1))
                        if tg % 2 == 0:
                            nc.scalar.copy(
                                stage[:, tg * 512:(tg + 1) * 512], pl[:])
                        else:
                            nc.vector.tensor_copy(
                                stage[:, tg * 512:(tg + 1) * 512], pl[:])
                    vg = ch * 4 + v
                    nc.sync.dma_start(
                        y_d[vg * 128:(vg + 1) * 128, :], stage[:])

    nc.compile()
    return nc


def _bf(a):
    return np.ascontiguousarray(a.astype(ml_dtypes.bfloat16))


def host_shards(inputs, debug=False):
    idx = np.asarray(inputs["idx"])
    wte = np.asarray(inputs["wte"], np.float32)
    wpe = np.asarray(inputs["wpe"], np.float32)
    ln1_g = np.asarray(inputs["ln1_g"], np.float32)
    ln2_g = np.asarray(inputs["ln2_g"], np.float32)
    lnf_g = np.asarray(inputs["lnf_g"], np.float32)
    for nm in ("ln1_b", "ln2_b", "fc_b", "mproj_b", "lnf_b"):
        assert np.abs(np.asarray(inputs[nm])).max() == 0.0, f"{nm} nonzero"
    LS = 32.0 / 8.0
    qs = 1.0 / math.sqrt(HD)

    aw = np.empty((L, C, 3 * C), np.float32)
    ala = np.empty((L, C, R), np.float32)
    alb = np.empty((L, R, 3 * C), np.float32)
    pw = np.empty((L, C, C), np.float32)
    pla = np.empty((L, C, R), np.float32)
    plb = np.empty((L, R, C), np.float32)
    fw = np.empty((L, C, 4 * C), np.float32)
    mw = np.empty((L, 4 * C, C), np.float32)
    for i in range(L):
        a = (np.asarray(inputs["attn_w"][i], np.float32) * ln1_g[i][None, :]).T
        a = a.copy()
        a[:, :C] *= qs
        aw[i] = a
        ala[i] = (np.asarray(inputs["attn_lA"][i], np.float32)
                  * ln1_g[i][None, :]).T
        b = np.asarray(inputs["attn_lB"][i], np.float32).T * LS
        b = b.copy()
        b[:, :C] *= qs
        alb[i] = b
        pw[i] = np.asarray(inputs["proj_w"][i], np.float32).T
        pla[i] = np.asarray(inputs["proj_lA"][i], np.float32).T
        plb[i] = np.asarray(inputs["proj_lB"][i], np.float32).T * LS
        fw[i] = (np.asarray(inputs["fc_w"][i], np.float32)
                 * ln2_g[i][None, :]).T
        mw[i] = np.asarray(inputs["mproj_w"][i], np.float32).T
    hwT = (np.asarray(inputs["head_w"], np.float32) * lnf_g[None, :]).T  # [C,V]

    common = dict(aw=_bf(aw), ala=_bf(ala), alb=_bf(alb), pw=_bf(pw),
                  pla=_bf(pla), plb=_bf(plb), fw=_bf(fw), mw=_bf(mw),
                  ident=_bf(np.eye(128, dtype=np.float32)))

    in_maps = []
    for c in range(NCORES):
        sl = slice(c * 128, (c + 1) * 128)
        x0 = np.stack([wte[idx[b2]][sl] + wpe[sl] for b2 in range(2)])
        x0 = np.ascontiguousarray(x0, np.float32)
        # maskT[kk, kc, qq]: 0 where key (kc*128+kk) <= query (c*128+qq)
        kidx = np.arange(128)[:, None, None] + 128 * np.arange(8)[None, :, None]
        qidx = c * 128 + np.arange(128)[None, None, :]
        maskT = _bf(np.where(kidx <= qidx, 0.0, NEG).astype(np.float32))
        hw = np.zeros((C, VCP), np.float32)
        lo, hi = c * VC, min((c + 1) * VC, V)
        hw[:, 0:hi - lo] = hwT[:, lo:hi]
        m = dict(common)
        m.update(x0=x0, maskT=maskT, hw=_bf(hw))
        in_maps.append(m)
    return in_maps


def kernel(**inputs):
    if "nc" not in _CACHE:
        _CACHE["nc"] = build_nc(debug=False)
    nc = _CACHE["nc"]
    in_maps = host_shards(inputs)
    res = bass_utils.run_bass_kernel_spmd(nc, in_maps,
                                          core_ids=list(range(NCORES)))
    out = np.empty((B * T, V), np.float32)
    for c in range(NCORES):
        lo, hi = c * VC, min((c + 1) * VC, V)
        out[:, lo:hi] = res.results[c]["y"][0:hi - lo, :].T.astype(np.float32)
    return out.reshape(B, T, V)
